# revision 1
# baseline (speedup 1.0000x reference)
"""Trainium2 Bass kernel for nn_ImagePatch: scatter-add 64 gathered 16x16x3
patches into each of 32 images [3,512,512], data-parallel over batch on 8 cores.

Band layout per (batch): partition p = c*32 + g  (c: channel, g: 16-row band).
Band g holds image rows [16g, 16g+16) as "primary" rows 0..15 plus 15 "slop"
rows 16..30 (zeroed) so a patch starting at row r (band g*=r>>4, rho=r&15)
always fits rows rho..rho+15 < 31 of one band.  After all patch adds, slop
rows of band g-1 are folded into primary rows of band g.  Patch placement
across the dynamic band index is done by a per-partition mask (PE broadcasts
the patch row to all 96 partitions, DVE does fused (patch*mask)+window).
"""

import numpy as np

import concourse.bacc as bacc
import concourse.bass as bass
import concourse.mybir as mybir
import concourse.tile as tile
from concourse.bass_utils import run_bass_kernel_spmd

B, N, C, H, W = 32, 64, 3, 512, 512
P = 16
NUM_CLASSES = 128
NCORES = 8
BPC = B // NCORES  # batches per core

G = 32          # bands per channel
BANDP = 3 * G   # 96 partitions used
ROWS = 31       # rows per band (16 primary + 15 slop)
PROW = ROWS * W     # elements per partition (15872)
PRIM = 16 * W       # primary region size (8192)


def build_nc():
    nc = bacc.Bacc("TRN2", target_bir_lowering=False, enable_asserts=False)
    img_d = nc.dram_tensor("image", [BPC, C, H, W], mybir.dt.float32, kind="ExternalInput")
    emb_d = nc.dram_tensor("emb", [NUM_CLASSES, 768], mybir.dt.float32, kind="ExternalInput")
    lab_d = nc.dram_tensor("labels", [BPC, N], mybir.dt.int32, kind="ExternalInput")
    tl_d = nc.dram_tensor("tl", [BPC, 2 * N], mybir.dt.int32, kind="ExternalInput")
    cg_d = nc.dram_tensor("const_g", [BANDP, 1], mybir.dt.int32, kind="ExternalInput")
    oneh_d = nc.dram_tensor("oneh", [64, 32 * 32], mybir.dt.float32, kind="ExternalInput")
    shf_d = nc.dram_tensor("shf", [BANDP, 2 * BANDP], mybir.dt.float32, kind="ExternalInput")
    out_d = nc.dram_tensor("out", [BPC, C, H, W], mybir.dt.float32, kind="ExternalOutput")

    AT = mybir.AluOpType
    with tile.TileContext(nc) as tc:
        with (
            tc.tile_pool(name="big", bufs=2) as bigp,
            tc.tile_pool(name="small", bufs=4) as smallp,
            tc.tile_pool(name="const", bufs=1) as constp,
            tc.tile_pool(name="psum", bufs=4, space="PSUM") as psp,
            tc.tile_pool(name="psumf", bufs=2, space="PSUM") as psfp,
        ):
            offregs = [nc.vector.alloc_register(f"offreg{i}") for i in range(8)]
            cg = constp.tile([BANDP, 1], mybir.dt.int32)
            oneh = constp.tile([64, 32 * 32], mybir.dt.float32)
            shf = constp.tile([BANDP, 2 * BANDP], mybir.dt.float32)
            nc.sync.dma_start(out=cg[:], in_=cg_d[:])
            nc.sync.dma_start(out=oneh[:], in_=oneh_d[:])
            nc.sync.dma_start(out=shf[:], in_=shf_d[:])

            for b in range(BPC):
                T = bigp.tile([BANDP, PROW], mybir.dt.float32, tag="T")
                Th = T[:].tensor
                Tv = T[:].rearrange("p (r w) -> p r w", w=W)

                # 1. load image primary rows; zero slop rows
                nc.sync.dma_start(
                    out=Tv[:, 0:16, :],
                    in_=img_d[b].rearrange("c (g r) w -> (c g) r w", g=G),
                )
                nc.gpsimd.memset(T[:, PRIM:PROW], 0)

                # 2. gather patch rows from emb: RAW[n, :] = emb[labels[n]*768 ...]
                lab = smallp.tile([N, 1], mybir.dt.int32, tag="lab")
                nc.sync.dma_start(out=lab[:], in_=lab_d[b, :, None])
                raw = smallp.tile([N, 768], mybir.dt.float32, tag="raw")
                nc.gpsimd.indirect_dma_start(
                    out=raw[:], out_offset=None,
                    in_=emb_d[:],
                    in_offset=bass.IndirectOffsetOnAxis(ap=lab[:, :1], axis=0),
                )

                # 3. band masks MSK[p=(c,g), n] = (g == top_left_r[n]>>4)
                rb = smallp.tile([BANDP, N], mybir.dt.int32, tag="rb")
                nc.sync.dma_start(
                    out=rb[:],
                    in_=bass.AP(tl_d[:].tensor, b * 2 * N, [[0, BANDP], [2, N]]),
                )
                gst = smallp.tile([BANDP, N], mybir.dt.int32, tag="gst")
                nc.vector.tensor_scalar(out=gst[:], in0=rb[:], scalar1=4, scalar2=None,
                                        op0=AT.logical_shift_right)
                msk = smallp.tile([BANDP, N], mybir.dt.float32, tag="msk")
                nc.vector.tensor_tensor(out=msk[:], in0=gst[:], in1=cg[:].to_broadcast([BANDP, N]),
                                        op=AT.is_equal)

                # 4. free-dim offsets OFF[n] = (r&15)*512 + c0
                cb1 = smallp.tile([1, N], mybir.dt.int32, tag="cb1")
                nc.sync.dma_start(out=cb1[:], in_=bass.AP(tl_d[:].tensor, b * 2 * N + 1, [[0, 1], [2, N]]))
                t1 = smallp.tile([1, N], mybir.dt.int32, tag="t1")
                nc.vector.tensor_scalar(out=t1[:], in0=gst[0:1, :], scalar1=16, scalar2=None, op0=AT.mult)
                t2 = smallp.tile([1, N], mybir.dt.int32, tag="t2")
                nc.vector.tensor_tensor(out=t2[:], in0=rb[0:1, :], in1=t1[:], op=AT.subtract)
                off = smallp.tile([1, N], mybir.dt.int32, tag="off")
                nc.vector.tensor_scalar(out=t2[:], in0=t2[:], scalar1=W, scalar2=None, op0=AT.mult)
                nc.vector.tensor_tensor(out=off[:], in0=t2[:], in1=cb1[:], op=AT.add)

                # 5. per-patch: PE broadcast into psum, DVE fused masked add
                for n in range(N):
                    half, nh = divmod(n, 32)
                    ps = psp.tile([BANDP, 256], mybir.dt.float32, tag="ps")
                    lhsT = oneh[half * 32:(half + 1) * 32, nh * 32:(nh + 1) * 32]
                    for c in range(C):
                        nc.tensor.matmul(
                            out=ps[c * 32:(c + 1) * 32, :],
                            lhsT=lhsT,
                            rhs=raw[half * 32:(half + 1) * 32, c * 256:(c + 1) * 256],
                            start=True, stop=True,
                        )
                    offreg = offregs[n % 8]
                    nc.vector.reg_load(offreg, off[0:1, n:n + 1])
                    win = bass.AP(Th, offreg, [[PROW, BANDP], [W, P], [1, P]])
                    nc.vector.scalar_tensor_tensor(
                        out=win, in0=ps[:], scalar=msk[:, n:n + 1], in1=win,
                        op0=AT.mult, op1=AT.add,
                    )

                # 6. fold slop of band g-1 into primary of band g:
                # PE: psF = SHIFT@slop_rows + I@prim_rows, ACT copies back to T.
                # (engines can't do partition-shifted adds; PE is the shifter)
                rows_left = 15
                r0 = 0
                while rows_left > 0:
                    nr = min(2, rows_left)
                    psf = psfp.tile([BANDP, 2 * 512], mybir.dt.float32, tag="psf")
                    for j in range(nr):
                        nc.tensor.matmul(
                            out=psf[:, j * 512:(j + 1) * 512],
                            lhsT=shf[:, 0:BANDP],
                            rhs=Tv[:, 16 + r0 + j, :],
                            start=True, stop=False,
                        )
                    for j in range(nr):
                        nc.tensor.matmul(
                            out=psf[:, j * 512:(j + 1) * 512],
                            lhsT=shf[:, BANDP:2 * BANDP],
                            rhs=Tv[:, r0 + j, :],
                            start=False, stop=True,
                        )
                    nc.scalar.activation(
                        out=T[:, r0 * W:(r0 + nr) * W],
                        in_=psf[:, 0:nr * 512],
                        func=mybir.ActivationFunctionType.Copy,
                    )
                    r0 += nr
                    rows_left -= nr

                # 7. store primary rows
                nc.sync.dma_start(
                    out=out_d[b].rearrange("c (g r) w -> (c g) r w", g=G),
                    in_=Tv[:, 0:16, :],
                )
    nc.finalize()
    return nc


def make_consts():
    cg = (np.arange(BANDP, dtype=np.int32) % G).reshape(BANDP, 1)
    oneh = np.zeros((64, 32 * 32), dtype=np.float32)
    for k in range(64):
        oneh[k, (k % 32) * 32:(k % 32) * 32 + 32] = 1.0
    # shf[:, :96] = SHIFT (out[m] = in[m-1]); shf[:, 96:] = identity
    shf = np.concatenate([np.eye(BANDP, BANDP, 1), np.eye(BANDP)], axis=1).astype(np.float32)
    return cg, oneh, shf


_NC_CACHE = {}


def kernel(image, emb, labels, top_left):
    image = np.ascontiguousarray(np.asarray(image), dtype=np.float32)
    emb_f = np.ascontiguousarray(np.asarray(emb), dtype=np.float32)
    labels = np.ascontiguousarray(np.asarray(labels)).astype(np.int32)
    tl = np.ascontiguousarray(np.asarray(top_left)).astype(np.int32)

    if "nc" not in _NC_CACHE:
        _NC_CACHE["nc"] = build_nc()
    nc = _NC_CACHE["nc"]

    cg, oneh, shf = make_consts()
    in_maps = []
    for k in range(NCORES):
        sl = slice(k * BPC, (k + 1) * BPC)
        in_maps.append({
            "image": image[sl],
            "emb": emb_f,
            "labels": labels[sl],
            "tl": tl[sl].reshape(BPC, 2 * N),
            "const_g": cg,
            "oneh": oneh,
            "shf": shf,
        })
    res = run_bass_kernel_spmd(nc, in_maps, core_ids=list(range(NCORES)))
    _NC_CACHE["last_res"] = res
    out = np.concatenate([r["out"] for r in res.results], axis=0)
    return out



# revision 29
# speedup vs baseline: 1.2705x; 1.2705x over previous
"""Trainium2 Bass kernel for nn_ImagePatch: scatter-add 64 gathered 16x16x3
patches into each of 32 images [3,512,512], data-parallel over batch on 8 cores.

Band layout per image: partition p = c*32 + g (c: channel, g: 16-row band).
Band g holds image rows [16g, 16g+16) as "primary" rows 0..15 plus 15 zeroed
"slop" rows so a patch starting at row r (band g*=r>>4, rho=r&15) always fits
rows rho..rho+15 of one band.  Patch placement across the dynamic band index
uses a per-partition mask; the patch is broadcast to all 96 partitions by one
fp32r PE matmul per 2 patches (channel-major gathered raw3 x constant
selector), and DVE does the fused (patch*mask)+window add at a register
offset.  Slop folds into the next band via a partition-shifted SBUF->SBUF
DMA accumulate (CCE add); slop re-zero is a DMA from a zero tile.
Index/mask/offset metadata is precomputed on host from labels/top_left.
"""

import numpy as np

import concourse.bacc as bacc
import concourse.bass as bass
import concourse.mybir as mybir
import concourse.tile as tile
from concourse.bass_utils import run_bass_kernel_spmd

B, N, C, H, W = 32, 64, 3, 512, 512
P = 16
NUM_CLASSES = 128
NCORES = 8
BPC = B // NCORES  # batches per core

G = 32          # bands per channel
BANDP = 3 * G   # 96 partitions used
ROWS = 31       # rows per band (16 primary + 15 slop)
PROW = ROWS * W     # elements per partition (15872)
PRIM = 16 * W       # primary region size (8192)
SLOP = PROW - PRIM  # slop region size (7680)


def build_nc():
    F32R = mybir.dt.float32r
    nc = bacc.Bacc("TRN2", target_bir_lowering=False, enable_asserts=False)
    img_d = nc.dram_tensor("image", [BPC, C, H, W], mybir.dt.float32, kind="ExternalInput")
    emb3_d = nc.dram_tensor("emb3", [NUM_CLASSES * 3, 256], mybir.dt.float32, kind="ExternalInput")
    lab3_d = nc.dram_tensor("lab3", [BPC, BANDP, 2], mybir.dt.int32, kind="ExternalInput")
    msk_d = nc.dram_tensor("mskh", [BPC, BANDP, N], mybir.dt.float32, kind="ExternalInput")
    off_d = nc.dram_tensor("offh", [BPC, N], mybir.dt.int32, kind="ExternalInput")
    sel_d = nc.dram_tensor("selm", [BANDP, 32 * BANDP], mybir.dt.float32, kind="ExternalInput")
    out_d = nc.dram_tensor("out", [BPC, C, H, W], mybir.dt.float32, kind="ExternalOutput")

    AT = mybir.AluOpType
    with tile.TileContext(nc) as tc:
        with (
            tc.tile_pool(name="big", bufs=2) as bigp,
            tc.tile_pool(name="small", bufs=4) as smallp,
            tc.tile_pool(name="const", bufs=1) as constp,
            tc.tile_pool(name="psum", bufs=4, space="PSUM") as psp,
            tc.tile_pool(name="fold", bufs=1) as foldp,
        ):
            offregs = [nc.vector.alloc_register(f"offreg{i}") for i in range(8)]
            selm = constp.tile([BANDP, 32 * BANDP], mybir.dt.float32)
            nc.sync.dma_start(out=selm[:], in_=sel_d[:])
            zt = constp.tile([BANDP, SLOP // 2], mybir.dt.float32)
            nc.gpsimd.memset(zt[:], 0)
            # fold staging: partition 0 stays zero forever (band 0 has no
            # predecessor); per-image DMAs only write partitions 1..95
            S = foldp.tile([BANDP, SLOP], mybir.dt.float32, tag="S")
            nc.gpsimd.memset(S[:], 0)

            for b in range(BPC):
                T = bigp.tile([BANDP, PROW], mybir.dt.float32, tag="T")
                Th = T[:].tensor
                Tv = T[:].rearrange("p (r w) -> p r w", w=W)

                # 1. load image primary rows; zero slop rows via DMA copy
                nc.sync.dma_start(
                    out=Tv[:, 0:16, :],
                    in_=img_d[b].rearrange("c (g r) w -> (c g) r w", g=G),
                )
                nc.scalar.dma_start(out=T[:, PRIM:PRIM + SLOP // 2], in_=zt[:])
                nc.scalar.dma_start(out=T[:, PRIM + SLOP // 2:PROW], in_=zt[:])

                # 2. channel-major gather: raw3[(c,m), h*256:...] = emb3[lab3[(c,m),h]]
                lab3 = smallp.tile([BANDP, 2], mybir.dt.int32, tag="lab3")
                nc.sync.dma_start(out=lab3[:], in_=lab3_d[b])
                raw3 = smallp.tile([BANDP, 512], mybir.dt.float32, tag="raw3")
                for h in range(2):
                    nc.gpsimd.indirect_dma_start(
                        out=raw3[:, h * 256:(h + 1) * 256], out_offset=None,
                        in_=emb3_d[:],
                        in_offset=bass.IndirectOffsetOnAxis(ap=lab3[:, h:h + 1], axis=0),
                    )

                # 3. host-precomputed band masks and free-dim offsets
                msk = smallp.tile([BANDP, N], mybir.dt.float32, tag="msk")
                nc.sync.dma_start(out=msk[:], in_=msk_d[b])
                off = smallp.tile([1, N], mybir.dt.int32, tag="off")
                nc.sync.dma_start(out=off[:], in_=off_d[b, None, :])

                # 4. per patch pair (m, m+32): one fp32r matmul broadcasts both
                # patches to all 96 partitions; DVE adds each masked window.
                # off/msk are host-reordered to pair order j = 2*m + h, so one
                # reg_load fills all 8 offset registers per 4 pairs.
                for m in range(32):
                    ps = psp.tile([BANDP, 512], mybir.dt.float32, tag="ps")
                    nc.tensor.matmul(
                        out=ps[:],
                        lhsT=selm[:, m * BANDP:(m + 1) * BANDP],
                        rhs=raw3[:],
                        start=True, stop=True,
                    )
                    if m % 4 == 0:
                        nc.vector.reg_load(offregs, off[0:1, 2 * m:2 * m + 8])
                    for h in range(2):
                        j = 2 * m + h
                        win = bass.AP(Th, offregs[j % 8], [[PROW, BANDP], [W, P], [1, P]])
                        nc.vector.scalar_tensor_tensor(
                            out=win, in0=ps[:, h * 256:(h + 1) * 256],
                            scalar=msk[:, j:j + 1], in1=win,
                            op0=AT.mult, op1=AT.add,
                        )

                # 5. fold: shift slop rows of band g-1 down one partition via
                # SBUF->SBUF DMA, then add into primary rows of band g on Pool.
                # Band 31's slop is always zero (top_left <= H-P), so the
                # cross-channel wrap at partitions 32/64 adds zeros: harmless.
                nc.sync.dma_start(out=S[1:BANDP, :], in_=T[0:BANDP - 1, PRIM:PROW])
                nc.gpsimd.tensor_tensor(
                    out=T[:, 0:SLOP], in0=S[:],
                    in1=T[:, 0:SLOP], op=AT.add,
                )

                # 6. store primary rows
                nc.scalar.dma_start(
                    out=out_d[b].rearrange("c (g r) w -> (c g) r w", g=G),
                    in_=Tv[:, 0:16, :],
                )
    nc.finalize()
    return nc


def make_selm():
    # selm[(c,m'), m*96 + (c*32+g)] = 1 iff m'==m (any g, channel-matched)
    sel = np.zeros((BANDP, 32 * BANDP), dtype=np.float32)
    for c in range(C):
        for m in range(32):
            sel[c * 32 + m, m * BANDP + c * 32:m * BANDP + c * 32 + 32] = 1.0
    return sel


_NC_CACHE = {}


def kernel(image, emb, labels, top_left):
    image = np.ascontiguousarray(np.asarray(image), dtype=np.float32)
    emb_f = np.ascontiguousarray(np.asarray(emb), dtype=np.float32)
    labels = np.ascontiguousarray(np.asarray(labels)).astype(np.int32)
    tl = np.ascontiguousarray(np.asarray(top_left)).astype(np.int64)

    if "nc" not in _NC_CACHE:
        _NC_CACHE["nc"] = build_nc()
    nc = _NC_CACHE["nc"]

    # host-side metadata: gather indices, band masks, window offsets
    emb3 = emb_f.reshape(NUM_CLASSES * 3, 256)
    labr = labels.reshape(B, 2, 32)                     # [B, h, m]
    lab3 = labr[:, None, :, :] * 3 + np.arange(C, dtype=np.int32)[None, :, None, None]
    lab3 = lab3.transpose(0, 1, 3, 2).reshape(B, BANDP, 2).astype(np.int32)
    r = tl[..., 0].astype(np.int32)                     # [B, N]
    c0 = tl[..., 1].astype(np.int32)
    g = r >> 4
    off = ((r & 15) << 9) + c0                          # rho*512 + c0
    gg = np.arange(G, dtype=np.int32)
    msk = (g[:, None, :] == gg[None, :, None]).astype(np.float32)   # [B,32,N]
    msk = np.tile(msk, (1, C, 1))                        # [B,96,N]
    # reorder patch metadata to pair order j = 2*m + h (n = h*32 + m)
    order = np.arange(N).reshape(2, 32).T.reshape(-1)    # j -> n
    off = off[:, order]
    msk = msk[:, :, order]
    selm = make_selm()

    in_maps = []
    for k in range(NCORES):
        sl = slice(k * BPC, (k + 1) * BPC)
        in_maps.append({
            "image": image[sl],
            "emb3": emb3,
            "lab3": lab3[sl],
            "mskh": msk[sl],
            "offh": off[sl],
            "selm": selm,
        })
    res = run_bass_kernel_spmd(nc, in_maps, core_ids=list(range(NCORES)))
    _NC_CACHE["last_res"] = res
    out = np.concatenate([r["out"] for r in res.results], axis=0)
    return out


# revision 30
# speedup vs baseline: 1.3511x; 1.0634x over previous
"""Trainium2 Bass kernel for nn_ImagePatch: scatter-add 64 gathered 16x16x3
patches into each of 32 images [3,512,512], data-parallel over batch on 8 cores.

Band layout per image: partition p = c*32 + g (c: channel, g: 16-row band).
Band g holds image rows [16g, 16g+16) as "primary" rows 0..15 plus 15 zeroed
"slop" rows so a patch starting at row r (band g*=r>>4, rho=r&15) always fits
rows rho..rho+15 of one band.  Patch placement across the dynamic band index
uses a per-partition mask; the patch is broadcast to all 96 partitions by one
fp32r PE matmul per 2 patches (channel-major gathered raw3 x constant
selector), and DVE does the fused (patch*mask)+window add at a register
offset.  Slop folds into the next band via a partition-shifted SBUF->SBUF
DMA accumulate (CCE add); slop re-zero is a DMA from a zero tile.
Index/mask/offset metadata is precomputed on host from labels/top_left.
"""

import numpy as np

import concourse.bacc as bacc
import concourse.bass as bass
import concourse.mybir as mybir
import concourse.tile as tile
from concourse.bass_utils import run_bass_kernel_spmd

B, N, C, H, W = 32, 64, 3, 512, 512
P = 16
NUM_CLASSES = 128
NCORES = 8
BPC = B // NCORES  # batches per core

G = 32          # bands per channel
BANDP = 3 * G   # 96 partitions used
ROWS = 31       # rows per band (16 primary + 15 slop)
PROW = ROWS * W     # elements per partition (15872)
PRIM = 16 * W       # primary region size (8192)
SLOP = PROW - PRIM  # slop region size (7680)


def build_nc():
    F32R = mybir.dt.float32r
    nc = bacc.Bacc("TRN2", target_bir_lowering=False, enable_asserts=False)
    img_d = nc.dram_tensor("image", [BPC, C, H, W], mybir.dt.float32, kind="ExternalInput")
    emb3_d = nc.dram_tensor("emb3", [NUM_CLASSES * 3, 256], mybir.dt.float32, kind="ExternalInput")
    lab3_d = nc.dram_tensor("lab3", [BPC, BANDP, 2], mybir.dt.int32, kind="ExternalInput")
    msk_d = nc.dram_tensor("mskh", [BPC, BANDP, N], mybir.dt.float32, kind="ExternalInput")
    off_d = nc.dram_tensor("offh", [BPC, N], mybir.dt.int32, kind="ExternalInput")
    sel_d = nc.dram_tensor("selm", [BANDP, 32 * BANDP], mybir.dt.float32, kind="ExternalInput")
    out_d = nc.dram_tensor("out", [BPC, C, H, W], mybir.dt.float32, kind="ExternalOutput")

    AT = mybir.AluOpType
    with tile.TileContext(nc) as tc:
        with (
            tc.tile_pool(name="big", bufs=2) as bigp,
            tc.tile_pool(name="small", bufs=4) as smallp,
            tc.tile_pool(name="const", bufs=1) as constp,
            tc.tile_pool(name="psum", bufs=8, space="PSUM") as psp,
            tc.tile_pool(name="fold", bufs=1) as foldp,
        ):
            offregs = [nc.vector.alloc_register(f"offreg{i}") for i in range(8)]
            selm = constp.tile([BANDP, 32 * BANDP], mybir.dt.float32)
            nc.sync.dma_start(out=selm[:], in_=sel_d[:])
            zt = constp.tile([BANDP, SLOP // 2], mybir.dt.float32)
            nc.gpsimd.memset(zt[:], 0)
            # fold staging: partition 0 stays zero forever (band 0 has no
            # predecessor); per-image DMAs only write partitions 1..95
            S = foldp.tile([BANDP, SLOP], mybir.dt.float32, tag="S")
            nc.gpsimd.memset(S[:], 0)

            for b in range(BPC):
                T = bigp.tile([BANDP, PROW], mybir.dt.float32, tag="T")
                Th = T[:].tensor
                Tv = T[:].rearrange("p (r w) -> p r w", w=W)

                # 1. load image primary rows; zero slop rows via DMA copy
                nc.sync.dma_start(
                    out=Tv[:, 0:16, :],
                    in_=img_d[b].rearrange("c (g r) w -> (c g) r w", g=G),
                )
                nc.scalar.dma_start(out=T[:, PRIM:PRIM + SLOP // 2], in_=zt[:])
                nc.scalar.dma_start(out=T[:, PRIM + SLOP // 2:PROW], in_=zt[:])

                # 2. channel-major gather: raw3[(c,m), h*256:...] = emb3[lab3[(c,m),h]]
                lab3 = smallp.tile([BANDP, 2], mybir.dt.int32, tag="lab3")
                nc.scalar.dma_start(out=lab3[:], in_=lab3_d[b])
                raw3 = smallp.tile([BANDP, 512], mybir.dt.float32, tag="raw3")
                for h in range(2):
                    nc.gpsimd.indirect_dma_start(
                        out=raw3[:, h * 256:(h + 1) * 256], out_offset=None,
                        in_=emb3_d[:],
                        in_offset=bass.IndirectOffsetOnAxis(ap=lab3[:, h:h + 1], axis=0),
                    )

                # 3. host-precomputed band masks and free-dim offsets
                msk = smallp.tile([BANDP, N], mybir.dt.float32, tag="msk")
                nc.scalar.dma_start(out=msk[:], in_=msk_d[b])
                off = smallp.tile([1, N], mybir.dt.int32, tag="off")
                nc.scalar.dma_start(out=off[:], in_=off_d[b, None, :])

                # 4. per patch pair (m, m+32): one fp32r matmul broadcasts both
                # patches to all 96 partitions; DVE adds each masked window.
                # off/msk are host-reordered to pair order j = 2*m + h, so one
                # reg_load fills all 8 offset registers per 4 pairs.
                for m in range(32):
                    ps = psp.tile([BANDP, 512], mybir.dt.float32, tag="ps")
                    nc.tensor.matmul(
                        out=ps[:],
                        lhsT=selm[:, m * BANDP:(m + 1) * BANDP],
                        rhs=raw3[:],
                        start=True, stop=True,
                    )
                    if m % 4 == 0:
                        nc.vector.reg_load(offregs, off[0:1, 2 * m:2 * m + 8])
                    for h in range(2):
                        j = 2 * m + h
                        win = bass.AP(Th, offregs[j % 8], [[PROW, BANDP], [W, P], [1, P]])
                        nc.vector.scalar_tensor_tensor(
                            out=win, in0=ps[:, h * 256:(h + 1) * 256],
                            scalar=msk[:, j:j + 1], in1=win,
                            op0=AT.mult, op1=AT.add,
                        )

                # 5. fold: shift slop rows of band g-1 down one partition via
                # SBUF->SBUF DMA, then add into primary rows of band g on Pool.
                # Band 31's slop is always zero (top_left <= H-P), so the
                # cross-channel wrap at partitions 32/64 adds zeros: harmless.
                nc.sync.dma_start(out=S[1:BANDP, :], in_=T[0:BANDP - 1, PRIM:PROW])
                nc.gpsimd.tensor_tensor(
                    out=T[:, 0:SLOP], in0=S[:],
                    in1=T[:, 0:SLOP], op=AT.add,
                )

                # 6. store primary rows
                nc.scalar.dma_start(
                    out=out_d[b].rearrange("c (g r) w -> (c g) r w", g=G),
                    in_=Tv[:, 0:16, :],
                )
    nc.finalize()
    return nc


def make_selm():
    # selm[(c,m'), m*96 + (c*32+g)] = 1 iff m'==m (any g, channel-matched)
    sel = np.zeros((BANDP, 32 * BANDP), dtype=np.float32)
    for c in range(C):
        for m in range(32):
            sel[c * 32 + m, m * BANDP + c * 32:m * BANDP + c * 32 + 32] = 1.0
    return sel


_NC_CACHE = {}


def kernel(image, emb, labels, top_left):
    image = np.ascontiguousarray(np.asarray(image), dtype=np.float32)
    emb_f = np.ascontiguousarray(np.asarray(emb), dtype=np.float32)
    labels = np.ascontiguousarray(np.asarray(labels)).astype(np.int32)
    tl = np.ascontiguousarray(np.asarray(top_left)).astype(np.int64)

    if "nc" not in _NC_CACHE:
        _NC_CACHE["nc"] = build_nc()
    nc = _NC_CACHE["nc"]

    # host-side metadata: gather indices, band masks, window offsets
    emb3 = emb_f.reshape(NUM_CLASSES * 3, 256)
    labr = labels.reshape(B, 2, 32)                     # [B, h, m]
    lab3 = labr[:, None, :, :] * 3 + np.arange(C, dtype=np.int32)[None, :, None, None]
    lab3 = lab3.transpose(0, 1, 3, 2).reshape(B, BANDP, 2).astype(np.int32)
    r = tl[..., 0].astype(np.int32)                     # [B, N]
    c0 = tl[..., 1].astype(np.int32)
    g = r >> 4
    off = ((r & 15) << 9) + c0                          # rho*512 + c0
    gg = np.arange(G, dtype=np.int32)
    msk = (g[:, None, :] == gg[None, :, None]).astype(np.float32)   # [B,32,N]
    msk = np.tile(msk, (1, C, 1))                        # [B,96,N]
    # reorder patch metadata to pair order j = 2*m + h (n = h*32 + m)
    order = np.arange(N).reshape(2, 32).T.reshape(-1)    # j -> n
    off = off[:, order]
    msk = msk[:, :, order]
    selm = make_selm()

    in_maps = []
    for k in range(NCORES):
        sl = slice(k * BPC, (k + 1) * BPC)
        in_maps.append({
            "image": image[sl],
            "emb3": emb3,
            "lab3": lab3[sl],
            "mskh": msk[sl],
            "offh": off[sl],
            "selm": selm,
        })
    res = run_bass_kernel_spmd(nc, in_maps, core_ids=list(range(NCORES)))
    _NC_CACHE["last_res"] = res
    out = np.concatenate([r["out"] for r in res.results], axis=0)
    return out
